# revision 14
# baseline (speedup 1.0000x reference)
"""Trainium2 Bass kernel for nn_PlanNotesProjection.

Math (per batch b):
  own_f   = ownership[b].astype(f32)             # (K=32, S=4096)
  summed  = own_f @ emb[b]                       # (K, H=2048)
  counts  = clip(own_f.sum(-1), min=1)           # (K,)
  pooled  = summed / counts[:, None]
  proj    = pooled @ W + bias                    # (K, D=1024)
  out[b]  = LayerNorm(proj) * gamma + beta       # eps=1e-5

Implementation detail: the pooling matmul is computed in TRANSPOSED
orientation, sumT[h, k] = sum_s emb[s, h] * own[k, s], with emb chunks
as the stationary operand. That lands the H dim on partitions, which is
exactly what the projection matmul needs as lhsT — no on-chip transpose
required. The 1/counts scaling commutes past the projection matmul, so
it is applied to proj instead of pooled.

Sharding: data-parallel over B across 8 cores (one batch per core);
W/b/gamma/beta replicated. Host pre-transposes+casts ownership[b] to
ownT (S, K) f32 so the device needs no bool handling.
"""

import sys
from contextlib import ExitStack

import numpy as np

sys.path.insert(0, "/opt/trn_rl_repo")

B, K, S, H, D = 8, 32, 4096, 2048, 1024
LN_EPS = 1e-5
P = 128
SC = S // P    # 32 contraction chunks for pooling
HC = H // P    # 16 h-tiles (partition tiles of H)
DJ = D // 512  # 2 psum column tiles for projection

TRACE = False
LAST_RESULT = None
_NC = None


def _build_nc(repeats=1):
    # repeats>1 unrolls the whole compute body (including DMAs) multiple
    # times in one NEFF; used by test.py to measure marginal per-iteration
    # HW time, cancelling host dispatch overhead. Grading uses repeats=1.
    import concourse.bass as bass
    import concourse.tile as tile
    from concourse import mybir
    from concourse.bacc import Bacc

    FP32 = mybir.dt.float32

    # Bacc (not plain Bass): its finalize() runs the legalization passes
    # (move_matmul_waits_to_ldweights, generate_event_semaphores) that split
    # multi-semaphore waits — TRN2 TPB instructions carry at most one.
    nc = Bacc("TRN2", target_bir_lowering=False)
    emb = nc.declare_dram_parameter("emb", [S, H], FP32, False)
    ownT = nc.declare_dram_parameter("ownT", [S, K], FP32, False)
    w = nc.declare_dram_parameter("w", [H, D], FP32, False)
    bvec = nc.declare_dram_parameter("bvec", [D], FP32, False)
    gamma = nc.declare_dram_parameter("gamma", [D], FP32, False)
    beta = nc.declare_dram_parameter("beta", [D], FP32, False)
    out = nc.declare_dram_parameter("out", [K, D], FP32, True)

    with ExitStack() as ctx:
        tc = ctx.enter_context(tile.TileContext(nc))

        own_pool = ctx.enter_context(tc.tile_pool(name="own", bufs=1))
        w_pool = ctx.enter_context(tc.tile_pool(name="w", bufs=1))
        emb_pool = ctx.enter_context(tc.tile_pool(name="emb", bufs=6))
        ones_pool = ctx.enter_context(tc.tile_pool(name="ones", bufs=1))
        eps_pool = ctx.enter_context(tc.tile_pool(name="eps", bufs=1))
        cnt_pool = ctx.enter_context(tc.tile_pool(name="cnt", bufs=1))
        st_pool = ctx.enter_context(tc.tile_pool(name="st", bufs=1))
        bc_pool = ctx.enter_context(tc.tile_pool(name="bc", bufs=1))
        x_pool = ctx.enter_context(tc.tile_pool(name="x", bufs=1))
        stats_pool = ctx.enter_context(tc.tile_pool(name="stats", bufs=1))
        mv_pool = ctx.enter_context(tc.tile_pool(name="mv", bufs=1))

        psum_sum = ctx.enter_context(tc.tile_pool(name="psum_sum", bufs=1, space="PSUM"))
        psum_proj = ctx.enter_context(tc.tile_pool(name="psum_proj", bufs=1, space="PSUM"))
        psum_cnt = ctx.enter_context(tc.tile_pool(name="psum_cnt", bufs=1, space="PSUM"))

        def body():
            # --- resident operands ---
            own_sb = own_pool.tile([P, SC, K], FP32)
            nc.sync.dma_start(own_sb[:], ownT.rearrange("(c p) k -> p c k", p=P))

            w_sb = w_pool.tile([P, HC, D], FP32)
            nc.sync.dma_start(w_sb[:], w.rearrange("(h p) d -> p h d", p=P))

            ones = ones_pool.tile([P, 1], FP32)
            nc.vector.memset(ones[:], 1.0)
            eps = eps_pool.tile([K, 1], FP32)
            nc.vector.memset(eps[:], LN_EPS)

            def bcast(vec):
                t = bc_pool.tile([K, D], FP32, name=f"bc_{vec.name}")
                ap = vec[:]
                bc_ap = bass.AP(tensor=ap.tensor, offset=ap.offset, ap=[[0, K]] + list(ap.ap))
                nc.gpsimd.dma_start(out=t[:], in_=bc_ap)
                return t

            bias_bc = bcast(bvec)
            gam_bc = bcast(gamma)
            bet_bc = bcast(beta)

            # --- pooling (transposed): sumT[h, k] = sum_s emb[s, h] own[k, s] ---
            # All 16 h-groups share ONE psum bank. A start=True matmul zeroes
            # the entire 2KB zero region (the whole bank), so per-group start
            # bits wipe each other out. Instead: memset the bank once, then
            # every matmul is a pure accumulate (start=False).
            sumT_ps = psum_sum.tile([P, HC, K], FP32)  # 2KB/partition = 1 bank
            cnt_ps = psum_cnt.tile([K, 1], FP32)
            nc.vector.memset(sumT_ps[:], 0.0)

            for c in range(SC):
                et = emb_pool.tile([P, H], FP32)
                nc.sync.dma_start(et[:], emb[c * P:(c + 1) * P, :])
                lhs_own = own_sb[:, c, :]
                nc.tensor.matmul(cnt_ps[:], lhs_own, ones[:], start=(c == 0), stop=(c == SC - 1))
                for h in range(HC):
                    nc.tensor.matmul(
                        sumT_ps[:, h, :], et[:, h * P:(h + 1) * P], lhs_own,
                        start=False, stop=False, skip_group_check=True,
                    )

            # --- counts -> 1/max(counts, 1) ---
            cnt_sb = cnt_pool.tile([K, 1], FP32)
            nc.vector.tensor_scalar_max(out=cnt_sb[:], in0=cnt_ps[:], scalar1=1.0)
            inv_sb = cnt_pool.tile([K, 1], FP32)
            nc.vector.reciprocal(out=inv_sb[:], in_=cnt_sb[:])

            # --- sumT PSUM -> SBUF (needed as matmul lhsT) ---
            sumT_sb = st_pool.tile([P, HC, K], FP32)
            nc.scalar.copy(out=sumT_sb[:], in_=sumT_ps[:])

            # --- projection: proj_raw[k, d] = sum_h sumT[h, k] W[h, d] ---
            proj_ps = [psum_proj.tile([K, 512], FP32, name=f"proj_ps{jj}") for jj in range(DJ)]
            for h in range(HC):
                for jj in range(DJ):
                    nc.tensor.matmul(
                        proj_ps[jj][:], sumT_sb[:, h, :], w_sb[:, h, jj * 512:(jj + 1) * 512],
                        start=(h == 0), stop=(h == HC - 1),
                    )

            # --- epilogue: x = proj_raw/counts + bias; LayerNorm; *gamma + beta ---
            x = x_pool.tile([K, D], FP32)
            for jj in range(DJ):
                nc.vector.tensor_scalar_mul(
                    out=x[:, jj * 512:(jj + 1) * 512], in0=proj_ps[jj][:], scalar1=inv_sb[:],
                )
            nc.vector.tensor_add(out=x[:], in0=x[:], in1=bias_bc[:])

            stats = stats_pool.tile([K, DJ, nc.vector.BN_STATS_DIM], FP32)
            for g in range(DJ):
                nc.vector.bn_stats(out=stats[:, g, :], in_=x[:, g * 512:(g + 1) * 512])
            mv = mv_pool.tile([K, nc.vector.BN_AGGR_DIM], FP32)
            nc.vector.bn_aggr(out=mv[:], in_=stats[:])
            nc.scalar.activation(
                out=mv[:, 1:2], in_=mv[:, 1:2],
                func=mybir.ActivationFunctionType.Sqrt, bias=eps[:], scale=1.0, alpha=0.0,
            )
            nc.vector.reciprocal(out=mv[:, 1:2], in_=mv[:, 1:2])
            normed = x_pool.tile([K, D], FP32)
            nc.vector.tensor_scalar(
                out=normed[:], in0=x[:], scalar1=mv[:, 0:1], scalar2=mv[:, 1:2],
                op0=mybir.AluOpType.subtract, op1=mybir.AluOpType.mult,
            )
            nc.vector.tensor_mul(out=normed[:], in0=normed[:], in1=gam_bc[:])
            outt = x_pool.tile([K, D], FP32)
            nc.vector.tensor_add(out=outt[:], in0=normed[:], in1=bet_bc[:])
            nc.sync.dma_start(out[:, :], outt[:])

        for _ in range(repeats):
            body()

    nc.finalize()
    return nc


def kernel(**inputs: np.ndarray) -> np.ndarray:
    global _NC, LAST_RESULT
    from concourse.bass_utils import run_bass_kernel_spmd

    emb = np.ascontiguousarray(np.asarray(inputs["plan_embeddings"], dtype=np.float32))
    own = np.asarray(inputs["ownership"])
    wmat = np.ascontiguousarray(np.asarray(inputs["W"], dtype=np.float32))
    bv = np.ascontiguousarray(np.asarray(inputs["b"], dtype=np.float32))
    ga = np.ascontiguousarray(np.asarray(inputs["gamma"], dtype=np.float32))
    be = np.ascontiguousarray(np.asarray(inputs["beta"], dtype=np.float32))

    if _NC is None:
        _NC = _build_nc()

    in_maps = []
    for i in range(B):
        in_maps.append({
            "emb": emb[i],
            "ownT": np.ascontiguousarray(own[i].T.astype(np.float32)),
            "w": wmat,
            "bvec": bv,
            "gamma": ga,
            "beta": be,
        })
    res = run_bass_kernel_spmd(_NC, in_maps, core_ids=list(range(B)), trace=TRACE)
    LAST_RESULT = res
    return np.stack([np.asarray(res.results[i]["out"]) for i in range(B)], axis=0).astype(np.float32)
